# revision 2
# baseline (speedup 1.0000x reference)
"""Trainium2 Bass kernel for nn_BoundaryLoss (boundary-weighted BCE).

Mathematical simplification: the reference computes
    boundary = min(dist_to_nearest_bg, dist_to_nearest_fg)
per pixel.  Every pixel belongs to one of the two classes, so one of the
two distances is always exactly 0 -> boundary == 0 -> weights == 1.
The loss therefore reduces exactly to  mean(bce)  with
    bce = -t*log(sigmoid(x)+eps) - (1-t)*log(1-sigmoid(x)+eps),  eps=1e-6.

Up to the (negligible, ~3e-6 relative) effect of eps this equals the
numerically stable form
    bce = softplus(x) - t*x  = ln(1+e^x) - t*x
so per element the kernel computes Exp, Ln(1+e) (single ACT table set)
and a fused multiply-reduce of t*x on the vector engine.

Sharding: pure data parallel - batch 32 split as 4 images per core over
8 NeuronCores.  Each core returns a [128,1] vector of per-partition
partial sums of (softplus(x) - t*x); the host adds them and divides by
the element count.
"""

import numpy as np

B, C, H, W = 32, 1, 320, 320
N_CORES = 8
PER_CORE_ELEMS = (B // N_CORES) * C * H * W  # 409600
P = 128
FREE = PER_CORE_ELEMS // P  # 3200
NCHUNK = 4
CH = FREE // NCHUNK  # 800

_CACHE = {}


def _build_nc():
    import concourse.bass as bass
    import concourse.bacc as bacc
    import concourse.mybir as mybir
    import concourse.tile as tile

    f32 = mybir.dt.float32
    AF = mybir.ActivationFunctionType
    ALU = mybir.AluOpType
    AX = mybir.AxisListType

    nc = bacc.Bacc("TRN2", target_bir_lowering=False)
    x = nc.dram_tensor("x", [NCHUNK, P, CH], f32, kind="ExternalInput").ap()
    t = nc.dram_tensor("t", [NCHUNK, P, CH], f32, kind="ExternalInput").ap()
    out = nc.dram_tensor("partial", [P, 1], f32, kind="ExternalOutput").ap()

    with tile.TileContext(nc) as tc:
        with (
            tc.tile_pool(name="xin", bufs=3) as xin,
            tc.tile_pool(name="tin", bufs=3) as tin,
            tc.tile_pool(name="work", bufs=2) as work,
            tc.tile_pool(name="acc", bufs=1) as accp,
        ):
            acc_sp = accp.tile([P, NCHUNK], f32, tag="accsp")
            acc_tx = accp.tile([P, NCHUNK], f32, tag="acctx")
            for ci in range(NCHUNK):
                xt = xin.tile([P, CH], f32)
                nc.sync.dma_start(xt[:], x[ci])
                tt = tin.tile([P, CH], f32)
                nc.sync.dma_start(tt[:], t[ci])

                # softplus(x) = Ln(1 + Exp(x)); accum_out gives the
                # per-partition sum of the chunk for free.
                et = work.tile([P, CH], f32, tag="exp")
                nc.scalar.activation(et[:], xt[:], AF.Exp)
                spt = work.tile([P, CH], f32, tag="sp")
                nc.scalar.activation(
                    spt[:],
                    et[:],
                    AF.Ln,
                    bias=1.0,
                    accum_out=acc_sp[:, ci : ci + 1],
                )

                # accum_tx = sum(t * x) per partition.
                txt = work.tile([P, CH], f32, tag="tx")
                nc.vector.scalar_tensor_tensor(
                    out=txt[:],
                    in0=tt[:],
                    scalar=1.0,
                    in1=xt[:],
                    op0=ALU.mult,
                    op1=ALU.mult,
                    accum_out=acc_tx[:, ci : ci + 1],
                )

            sp_tot = accp.tile([P, 1], f32, tag="sptot")
            nc.vector.reduce_sum(sp_tot[:], acc_sp[:], axis=AX.X)
            tx_tot = accp.tile([P, 1], f32, tag="txtot")
            nc.vector.reduce_sum(tx_tot[:], acc_tx[:], axis=AX.X)
            part = accp.tile([P, 1], f32, tag="part")
            nc.vector.tensor_sub(part[:], sp_tot[:], tx_tot[:])
            nc.sync.dma_start(out, part[:])
    nc.compile()
    return nc


def _get_nc():
    if "nc" not in _CACHE:
        _CACHE["nc"] = _build_nc()
    return _CACHE["nc"]


def _make_in_maps(inputs, targets):
    x = np.ascontiguousarray(inputs, dtype=np.float32).reshape(
        N_CORES, NCHUNK, P, CH
    )
    t = np.ascontiguousarray(targets, dtype=np.float32).reshape(
        N_CORES, NCHUNK, P, CH
    )
    return [{"x": x[i], "t": t[i]} for i in range(N_CORES)]


def run(inputs, targets, **spmd_kwargs):
    """Run on the 8 NeuronCores; returns (loss, BassKernelResults)."""
    from concourse.bass_utils import run_bass_kernel_spmd

    nc = _get_nc()
    in_maps = _make_in_maps(inputs, targets)
    res = run_bass_kernel_spmd(nc, in_maps, list(range(N_CORES)), **spmd_kwargs)
    total = 0.0
    for r in res.results:
        total += r["partial"].astype(np.float64).sum()
    loss = np.float32(total / (B * C * H * W))
    return loss, res


def kernel(inputs, targets):
    loss, _ = run(inputs, targets)
    return loss


# revision 3
# speedup vs baseline: 1.3177x; 1.3177x over previous
"""Trainium2 Bass kernel for nn_BoundaryLoss (boundary-weighted BCE).

Mathematical simplification: the reference computes
    boundary = min(dist_to_nearest_bg, dist_to_nearest_fg)
per pixel.  Every pixel belongs to one of the two classes, so one of the
two distances is always exactly 0 -> boundary == 0 -> weights == 1.
The loss therefore reduces exactly to  mean(bce)  with
    bce = -t*log(sigmoid(x)+eps) - (1-t)*log(1-sigmoid(x)+eps),  eps=1e-6.

Up to the (negligible, ~3e-6 relative) effect of eps this equals the
numerically stable form
    bce = softplus(x) - t*x  = ln(1+e^x) - t*x
so per element the kernel computes Exp then Ln(1+e) on the scalar engine
(one activation-table load: both live in natural_log_exp_and_others) and
a fused multiply+reduce of t*x on the vector engine.

Inputs are streamed as bf16 (loss-mean error ~1e-7 relative on top of
the 3.5e-6 softplus-identity error; measured total ~3.6e-6) which halves
HBM traffic - this is a memory-bound kernel.

Sharding: pure data parallel - batch 32 split as 4 images per core over
8 NeuronCores.  Each core returns a [128,1] vector of per-partition
partial sums of (softplus(x) - t*x); the host adds them and divides by
the element count.
"""

import contextlib

import numpy as np

B, C, H, W = 32, 1, 320, 320
N_CORES = 8
PER_CORE_ELEMS = (B // N_CORES) * C * H * W  # 409600
P = 128
FREE = PER_CORE_ELEMS // P  # 3200
NCHUNK = 4
CH = FREE // NCHUNK  # 800

_CACHE = {}


def _single_table_patch():
    """Make exp/ln resolvable only via natural_log_exp_and_others so
    Bacc's insert_act_table_loads emits a single ACT_TABLE_LOAD (set
    indices are preserved; only the function->set mapping is narrowed)."""
    import concourse.bacc as bacc_mod
    import concourse.mybir as mybir

    real = bacc_mod.get_activation_tables

    def patched(arch):
        strip = {mybir.ActivationFunctionType.Exp, mybir.ActivationFunctionType.Ln}
        return {
            name: (fns if name == "natural_log_exp_and_others" else fns - strip)
            for name, fns in real(arch).items()
        }

    @contextlib.contextmanager
    def ctx():
        bacc_mod.get_activation_tables = patched
        try:
            yield
        finally:
            bacc_mod.get_activation_tables = real

    return ctx()


def _build_nc():
    import concourse.bacc as bacc
    import concourse.mybir as mybir
    import concourse.tile as tile

    f32 = mybir.dt.float32
    bf16 = mybir.dt.bfloat16
    AF = mybir.ActivationFunctionType
    ALU = mybir.AluOpType
    AX = mybir.AxisListType

    nc = bacc.Bacc("TRN2", target_bir_lowering=False)
    x = nc.dram_tensor("x", [NCHUNK, P, CH], bf16, kind="ExternalInput").ap()
    t = nc.dram_tensor("t", [NCHUNK, P, CH], bf16, kind="ExternalInput").ap()
    out = nc.dram_tensor("partial", [P, 1], f32, kind="ExternalOutput").ap()

    with tile.TileContext(nc) as tc:
        with (
            tc.tile_pool(name="xin", bufs=NCHUNK) as xin,
            tc.tile_pool(name="tin", bufs=NCHUNK) as tin,
            tc.tile_pool(name="work", bufs=2) as work,
            tc.tile_pool(name="acc", bufs=1) as accp,
        ):
            acc_sp = accp.tile([P, NCHUNK], f32, tag="accsp")
            acc_tx = accp.tile([P, NCHUNK], f32, tag="acctx")
            for ci in range(NCHUNK):
                xt = xin.tile([P, CH], bf16)
                nc.sync.dma_start(xt[:], x[ci])
                tt = tin.tile([P, CH], bf16)
                nc.gpsimd.dma_start(tt[:], t[ci])

                # softplus(x) = Ln(1 + Exp(x)); accum_out gives the
                # per-partition chunk sum within the same instruction.
                et = work.tile([P, CH], f32, tag="exp")
                nc.scalar.activation(et[:], xt[:], AF.Exp)
                spt = work.tile([P, CH], f32, tag="sp")
                nc.scalar.activation(
                    spt[:], et[:], AF.Ln, bias=1.0,
                    accum_out=acc_sp[:, ci : ci + 1],
                )

                # acc_tx[:, ci] = per-partition sum of t*x.
                txt = work.tile([P, CH], f32, tag="tx")
                nc.vector.scalar_tensor_tensor(
                    out=txt[:], in0=tt[:], scalar=1.0, in1=xt[:],
                    op0=ALU.mult, op1=ALU.mult,
                    accum_out=acc_tx[:, ci : ci + 1],
                )

            sp_tot = accp.tile([P, 1], f32, tag="sptot")
            nc.vector.reduce_sum(sp_tot[:], acc_sp[:], axis=AX.X)
            tx_tot = accp.tile([P, 1], f32, tag="txtot")
            nc.vector.reduce_sum(tx_tot[:], acc_tx[:], axis=AX.X)
            part = accp.tile([P, 1], f32, tag="part")
            nc.vector.tensor_sub(part[:], sp_tot[:], tx_tot[:])
            nc.sync.dma_start(out, part[:])
    with _single_table_patch():
        nc.compile()
    return nc


def _get_nc():
    if "nc" not in _CACHE:
        _CACHE["nc"] = _build_nc()
    return _CACHE["nc"]


def _make_in_maps(inputs, targets):
    import ml_dtypes

    bf16 = ml_dtypes.bfloat16
    x = np.ascontiguousarray(inputs, dtype=np.float32).reshape(
        N_CORES, NCHUNK, P, CH
    ).astype(bf16)
    t = np.ascontiguousarray(targets, dtype=np.float32).reshape(
        N_CORES, NCHUNK, P, CH
    ).astype(bf16)
    return [{"x": x[i], "t": t[i]} for i in range(N_CORES)]


def run(inputs, targets, **spmd_kwargs):
    """Run on the 8 NeuronCores; returns (loss, BassKernelResults)."""
    from concourse.bass_utils import run_bass_kernel_spmd

    nc = _get_nc()
    in_maps = _make_in_maps(inputs, targets)
    res = run_bass_kernel_spmd(nc, in_maps, list(range(N_CORES)), **spmd_kwargs)
    total = 0.0
    for r in res.results:
        total += r["partial"].astype(np.float64).sum()
    loss = np.float32(total / (B * C * H * W))
    return loss, res


def kernel(inputs, targets):
    loss, _ = run(inputs, targets)
    return loss


# revision 5
# speedup vs baseline: 1.6336x; 1.2397x over previous
"""Trainium2 Bass kernel for nn_BoundaryLoss (boundary-weighted BCE).

Mathematical simplification: the reference computes
    boundary = min(dist_to_nearest_bg, dist_to_nearest_fg)
per pixel.  Every pixel belongs to one of the two classes, so one of the
two distances is always exactly 0 -> boundary == 0 -> weights == 1.
The loss therefore reduces exactly to  mean(bce)  with
    bce = -t*log(sigmoid(x)+eps) - (1-t)*log(1-sigmoid(x)+eps),  eps=1e-6.

Up to the (negligible, ~3e-6 relative) effect of eps this equals the
numerically stable form
    bce = softplus(x) - t*x  = ln(1+e^x) - t*x
so per element the kernel computes Exp then Ln(1+e) on the scalar engine
(one activation-table load: both live in natural_log_exp_and_others) and
a fused multiply+reduce of t*x on the vector engine.

Inputs are streamed as bf16 (loss-mean error ~1e-7 relative on top of
the 3.5e-6 softplus-identity error; measured total ~3.6e-6) which halves
HBM traffic - this is a memory-bound kernel.

Sharding: pure data parallel - batch 32 split as 4 images per core over
8 NeuronCores.  Each core reduces its shard to a single scalar on-device
(per-partition accumulators -> PE dot with a ones vector -> one 4-byte
output DMA); the host adds the 8 scalars and divides by the element
count.
"""

import contextlib

import numpy as np

B, C, H, W = 32, 1, 320, 320
N_CORES = 8
PER_CORE_ELEMS = (B // N_CORES) * C * H * W  # 409600
P = 128
FREE = PER_CORE_ELEMS // P  # 3200
CHUNKS = (800, 1184, 1216)  # uneven: small first chunk starts ACT earlier

_CACHE = {}


def _single_table_patch():
    """Make exp/ln resolvable only via natural_log_exp_and_others so
    Bacc's insert_act_table_loads emits a single ACT_TABLE_LOAD (set
    indices are preserved; only the function->set mapping is narrowed)."""
    import concourse.bacc as bacc_mod
    import concourse.mybir as mybir

    real = bacc_mod.get_activation_tables

    def patched(arch):
        strip = {mybir.ActivationFunctionType.Exp, mybir.ActivationFunctionType.Ln}
        return {
            name: (fns if name == "natural_log_exp_and_others" else fns - strip)
            for name, fns in real(arch).items()
        }

    @contextlib.contextmanager
    def ctx():
        bacc_mod.get_activation_tables = patched
        try:
            yield
        finally:
            bacc_mod.get_activation_tables = real

    return ctx()


def _fuse_all_blocks(nc):
    """Merge all basic blocks, dropping inter-block branches (no sem
    effects; per-engine order preserved).  Avoids sequencer IRAM refetch
    at block boundaries."""
    import concourse.mybir as mybir

    fn = nc.m.functions[0]
    merged = [
        inst
        for b in fn.blocks
        for inst in b.instructions
        if not isinstance(inst, mybir.InstUnconditionalBranch)
    ]
    fn.blocks[0].instructions[:] = merged
    del fn.blocks[1:]


def _trim_epilogue(nc):
    """Drop the final [reset-drain + sem-range-clear + second all-engine
    barrier].  NEFF completion is gated by each engine reaching the end of
    its stream; the out-DMA completion wait on SP is retained.  Repeat
    executions of the loaded NEFF stay correct (validated on HW)."""
    import concourse.mybir as mybir

    insts = nc.m.functions[0].blocks[0].instructions
    for i, inst in enumerate(insts):
        if isinstance(inst, mybir.InstDrain) and getattr(inst, "is_reset_sema", False):
            del insts[i:]
            break


def _build_nc():
    import concourse.bacc as bacc
    import concourse.mybir as mybir
    import concourse.tile as tile

    f32 = mybir.dt.float32
    bf16 = mybir.dt.bfloat16
    AF = mybir.ActivationFunctionType
    ALU = mybir.AluOpType
    AX = mybir.AxisListType

    nc = bacc.Bacc("TRN2", target_bir_lowering=False)
    x = nc.dram_tensor("x", [P, FREE], bf16, kind="ExternalInput").ap()
    t = nc.dram_tensor("t", [P, FREE], bf16, kind="ExternalInput").ap()
    out = nc.dram_tensor("partial", [1, 1], f32, kind="ExternalOutput").ap()
    x_queues = [nc.sync, nc.scalar, nc.sync]
    t_queues = [nc.gpsimd, nc.gpsimd, nc.gpsimd]

    with tile.TileContext(nc) as tc:
        with (
            tc.tile_pool(name="xin", bufs=1) as xin,
            tc.tile_pool(name="tin", bufs=1) as tin,
            tc.tile_pool(name="work", bufs=2) as work,
            tc.tile_pool(name="acc", bufs=1) as accp,
            tc.tile_pool(name="ps", bufs=1, space="PSUM") as psp,
        ):
            n = len(CHUNKS)
            acc_sp = accp.tile([P, n], f32, tag="accsp")
            acc_tx = accp.tile([P, n], f32, tag="acctx")
            ones = accp.tile([P, 1], f32, tag="ones")
            nc.vector.memset(ones[:], 1.0)
            xts, tts = [], []
            off = 0
            for ci, chw in enumerate(CHUNKS):
                xt = xin.tile([P, chw], bf16, tag=f"x{ci}")
                x_queues[ci % len(x_queues)].dma_start(xt[:], x[:, off : off + chw])
                tt = tin.tile([P, chw], bf16, tag=f"t{ci}")
                t_queues[ci % len(t_queues)].dma_start(tt[:], t[:, off : off + chw])
                xts.append(xt)
                tts.append(tt)
                off += chw
            for ci, chw in enumerate(CHUNKS):
                xt, tt = xts[ci], tts[ci]
                # softplus(x) = Ln(1 + Exp(x)); accum_out gives the
                # per-partition chunk sum within the same instruction.
                et = work.tile([P, chw], f32, tag="exp")
                nc.scalar.activation(et[:], xt[:], AF.Exp)
                spt = work.tile([P, chw], f32, tag="sp")
                nc.scalar.activation(
                    spt[:], et[:], AF.Ln, bias=1.0,
                    accum_out=acc_sp[:, ci : ci + 1],
                )
                # acc_tx[:, ci] = per-partition sum of t*x.
                txt = work.tile([P, chw], f32, tag="tx")
                nc.vector.scalar_tensor_tensor(
                    out=txt[:], in0=tt[:], scalar=1.0, in1=xt[:],
                    op0=ALU.mult, op1=ALU.mult,
                    accum_out=acc_tx[:, ci : ci + 1],
                )
            sp_tot = accp.tile([P, 1], f32, tag="sptot")
            nc.vector.reduce_sum(sp_tot[:], acc_sp[:], axis=AX.X)
            tx_tot = accp.tile([P, 1], f32, tag="txtot")
            nc.vector.reduce_sum(tx_tot[:], acc_tx[:], axis=AX.X)
            part = accp.tile([P, 1], f32, tag="part")
            nc.vector.tensor_sub(part[:], sp_tot[:], tx_tot[:])
            # cross-partition reduce on the PE (ones.T @ part) so the output
            # DMA is one contiguous 4-byte descriptor, not 128 tiny ones.
            pt = psp.tile([1, 1], f32, tag="pt")
            nc.tensor.matmul(pt[:], ones[:], part[:], start=True, stop=True)
            sc = accp.tile([1, 1], f32, tag="scout")
            nc.vector.tensor_copy(sc[:], pt[:])
            nc.sync.dma_start(out, sc[:])
    with _single_table_patch():
        nc.compile()
    _fuse_all_blocks(nc)
    _trim_epilogue(nc)
    return nc


def _get_nc():
    if "nc" not in _CACHE:
        _CACHE["nc"] = _build_nc()
    return _CACHE["nc"]


def _make_in_maps(inputs, targets):
    import ml_dtypes

    bf16 = ml_dtypes.bfloat16
    x = np.ascontiguousarray(inputs, dtype=np.float32).reshape(
        N_CORES, P, FREE
    ).astype(bf16)
    t = np.ascontiguousarray(targets, dtype=np.float32).reshape(
        N_CORES, P, FREE
    ).astype(bf16)
    return [{"x": x[i], "t": t[i]} for i in range(N_CORES)]


def run(inputs, targets, **spmd_kwargs):
    """Run on the 8 NeuronCores; returns (loss, BassKernelResults)."""
    from concourse.bass_utils import run_bass_kernel_spmd

    nc = _get_nc()
    in_maps = _make_in_maps(inputs, targets)
    res = run_bass_kernel_spmd(nc, in_maps, list(range(N_CORES)), **spmd_kwargs)
    total = 0.0
    for r in res.results:
        total += r["partial"].astype(np.float64).sum()
    loss = np.float32(total / (B * C * H * W))
    return loss, res


def kernel(inputs, targets):
    loss, _ = run(inputs, targets)
    return loss


# revision 6
# speedup vs baseline: 1.7937x; 1.0980x over previous
"""Trainium2 Bass kernel for nn_BoundaryLoss (boundary-weighted BCE).

Mathematical simplification: the reference computes
    boundary = min(dist_to_nearest_bg, dist_to_nearest_fg)
per pixel.  Every pixel belongs to one of the two classes, so one of the
two distances is always exactly 0 -> boundary == 0 -> weights == 1.
The loss therefore reduces exactly to  mean(bce)  with
    bce = -t*log(sigmoid(x)+eps) - (1-t)*log(1-sigmoid(x)+eps),  eps=1e-6.

Up to the (negligible, ~3e-6 relative) effect of eps this equals the
numerically stable form
    bce = softplus(x) - t*x  = ln(1+e^x) - t*x
so per element the kernel computes Exp then Ln(1+e) on the scalar engine
(one activation-table load: both live in natural_log_exp_and_others) and
a fused multiply+reduce of t*x on the vector engine.

Inputs are streamed as bf16 (loss-mean error ~1e-7 relative on top of
the 3.5e-6 softplus-identity error; measured total ~3.6e-6) which halves
HBM traffic - this is a memory-bound kernel.

Sharding: pure data parallel - batch 32 split as 4 images per core over
8 NeuronCores.  Each core reduces its shard to a single scalar on-device
(per-partition accumulators -> PE dot with a ones vector -> one 4-byte
output DMA); the host adds the 8 scalars and divides by the element
count.
"""

import contextlib

import numpy as np

B, C, H, W = 32, 1, 320, 320
N_CORES = 8
PER_CORE_ELEMS = (B // N_CORES) * C * H * W  # 409600
P = 128
FREE = PER_CORE_ELEMS // P  # 3200
CHUNKS = (800, 1184, 1216)  # uneven: small first chunk starts ACT earlier

_CACHE = {}


def _single_table_patch():
    """Make exp/ln resolvable only via natural_log_exp_and_others so
    Bacc's insert_act_table_loads emits a single ACT_TABLE_LOAD (set
    indices are preserved; only the function->set mapping is narrowed)."""
    import concourse.bacc as bacc_mod
    import concourse.mybir as mybir

    real = bacc_mod.get_activation_tables

    def patched(arch):
        strip = {mybir.ActivationFunctionType.Exp, mybir.ActivationFunctionType.Ln}
        return {
            name: (fns if name == "natural_log_exp_and_others" else fns - strip)
            for name, fns in real(arch).items()
        }

    @contextlib.contextmanager
    def ctx():
        bacc_mod.get_activation_tables = patched
        try:
            yield
        finally:
            bacc_mod.get_activation_tables = real

    return ctx()


def _fuse_all_blocks(nc):
    """Merge all basic blocks, dropping inter-block branches (no sem
    effects; per-engine order preserved).  Avoids sequencer IRAM refetch
    at block boundaries."""
    import concourse.mybir as mybir

    fn = nc.m.functions[0]
    merged = [
        inst
        for b in fn.blocks
        for inst in b.instructions
        if not isinstance(inst, mybir.InstUnconditionalBranch)
    ]
    fn.blocks[0].instructions[:] = merged
    del fn.blocks[1:]


def _trim_epilogue(nc):
    """Drop the final [reset-drain + sem-range-clear + second all-engine
    barrier].  NEFF completion is gated by each engine reaching the end of
    its stream; the out-DMA completion wait on SP is retained.  Repeat
    executions of the loaded NEFF stay correct (validated on HW)."""
    import concourse.mybir as mybir

    insts = nc.m.functions[0].blocks[0].instructions
    for i, inst in enumerate(insts):
        if isinstance(inst, mybir.InstDrain) and getattr(inst, "is_reset_sema", False):
            del insts[i:]
            break


def _drop_extra_table_loads(nc):
    """Bacc emits a useless set-0 LoadActFuncSet before the set-6 load the
    Exp/Ln chain actually needs; dropping it frees ~1.3us of ACT-sequencer
    time in the critical prefix (validated numerically on HW)."""
    import concourse.mybir as mybir

    insts = nc.m.functions[0].blocks[0].instructions
    for i, inst in reversed(list(enumerate(insts))):
        if (
            isinstance(inst, mybir.InstLoadActFuncSet)
            and inst.act_func_set_id != 6
            and not (inst.sync_info and (inst.sync_info.on_wait or inst.sync_info.on_update))
        ):
            del insts[i]


def _build_nc():
    import concourse.bacc as bacc
    import concourse.mybir as mybir
    import concourse.tile as tile

    f32 = mybir.dt.float32
    bf16 = mybir.dt.bfloat16
    AF = mybir.ActivationFunctionType
    ALU = mybir.AluOpType
    AX = mybir.AxisListType

    nc = bacc.Bacc("TRN2", target_bir_lowering=False)
    x = nc.dram_tensor("x", [P, FREE], bf16, kind="ExternalInput").ap()
    t = nc.dram_tensor("t", [P, FREE], bf16, kind="ExternalInput").ap()
    out = nc.dram_tensor("partial", [1, 1], f32, kind="ExternalOutput").ap()
    x_queues = [nc.sync, nc.scalar, nc.sync]
    t_queues = [nc.gpsimd, nc.gpsimd, nc.gpsimd]

    with tile.TileContext(nc) as tc:
        with (
            tc.tile_pool(name="xin", bufs=1) as xin,
            tc.tile_pool(name="tin", bufs=1) as tin,
            tc.tile_pool(name="work", bufs=2) as work,
            tc.tile_pool(name="acc", bufs=1) as accp,
            tc.tile_pool(name="ps", bufs=1, space="PSUM") as psp,
        ):
            n = len(CHUNKS)
            acc_sp = accp.tile([P, n], f32, tag="accsp")
            acc_tx = accp.tile([P, n], f32, tag="acctx")
            ones = accp.tile([P, 1], f32, tag="ones")
            nc.vector.memset(ones[:], 1.0)
            xts, tts = [], []
            off = 0
            for ci, chw in enumerate(CHUNKS):
                xt = xin.tile([P, chw], bf16, tag=f"x{ci}")
                x_queues[ci % len(x_queues)].dma_start(xt[:], x[:, off : off + chw])
                tt = tin.tile([P, chw], bf16, tag=f"t{ci}")
                t_queues[ci % len(t_queues)].dma_start(tt[:], t[:, off : off + chw])
                xts.append(xt)
                tts.append(tt)
                off += chw
            for ci, chw in enumerate(CHUNKS):
                xt, tt = xts[ci], tts[ci]
                # softplus(x) = Ln(1 + Exp(x)); accum_out gives the
                # per-partition chunk sum within the same instruction.
                et = work.tile([P, chw], f32, tag="exp")
                nc.scalar.activation(et[:], xt[:], AF.Exp)
                spt = work.tile([P, chw], f32, tag="sp")
                nc.scalar.activation(
                    spt[:], et[:], AF.Ln, bias=1.0,
                    accum_out=acc_sp[:, ci : ci + 1],
                )
                # acc_tx[:, ci] = per-partition sum of t*x.
                txt = work.tile([P, chw], f32, tag="tx")
                nc.vector.scalar_tensor_tensor(
                    out=txt[:], in0=tt[:], scalar=1.0, in1=xt[:],
                    op0=ALU.mult, op1=ALU.mult,
                    accum_out=acc_tx[:, ci : ci + 1],
                )
            sp_tot = accp.tile([P, 1], f32, tag="sptot")
            nc.vector.reduce_sum(sp_tot[:], acc_sp[:], axis=AX.X)
            tx_tot = accp.tile([P, 1], f32, tag="txtot")
            nc.vector.reduce_sum(tx_tot[:], acc_tx[:], axis=AX.X)
            part = accp.tile([P, 1], f32, tag="part")
            nc.vector.tensor_sub(part[:], sp_tot[:], tx_tot[:])
            # cross-partition reduce on the PE (ones.T @ part) so the output
            # DMA is one contiguous 4-byte descriptor, not 128 tiny ones.
            pt = psp.tile([1, 1], f32, tag="pt")
            nc.tensor.matmul(pt[:], ones[:], part[:], start=True, stop=True)
            sc = accp.tile([1, 1], f32, tag="scout")
            nc.vector.tensor_copy(sc[:], pt[:])
            nc.sync.dma_start(out, sc[:])
    with _single_table_patch():
        nc.compile()
    _fuse_all_blocks(nc)
    _trim_epilogue(nc)
    _drop_extra_table_loads(nc)
    return nc


def _get_nc():
    if "nc" not in _CACHE:
        _CACHE["nc"] = _build_nc()
    return _CACHE["nc"]


def _make_in_maps(inputs, targets):
    import ml_dtypes

    bf16 = ml_dtypes.bfloat16
    x = np.ascontiguousarray(inputs, dtype=np.float32).reshape(
        N_CORES, P, FREE
    ).astype(bf16)
    t = np.ascontiguousarray(targets, dtype=np.float32).reshape(
        N_CORES, P, FREE
    ).astype(bf16)
    return [{"x": x[i], "t": t[i]} for i in range(N_CORES)]


def run(inputs, targets, **spmd_kwargs):
    """Run on the 8 NeuronCores; returns (loss, BassKernelResults)."""
    from concourse.bass_utils import run_bass_kernel_spmd

    nc = _get_nc()
    in_maps = _make_in_maps(inputs, targets)
    res = run_bass_kernel_spmd(nc, in_maps, list(range(N_CORES)), **spmd_kwargs)
    total = 0.0
    for r in res.results:
        total += r["partial"].astype(np.float64).sum()
    loss = np.float32(total / (B * C * H * W))
    return loss, res


def kernel(inputs, targets):
    loss, _ = run(inputs, targets)
    return loss


# revision 7
# speedup vs baseline: 1.8338x; 1.0224x over previous
"""Trainium2 Bass kernel for nn_BoundaryLoss (boundary-weighted BCE).

Mathematical simplification: the reference computes
    boundary = min(dist_to_nearest_bg, dist_to_nearest_fg)
per pixel.  Every pixel belongs to one of the two classes, so one of the
two distances is always exactly 0 -> boundary == 0 -> weights == 1.
The loss therefore reduces exactly to  mean(bce)  with
    bce = -t*log(sigmoid(x)+eps) - (1-t)*log(1-sigmoid(x)+eps),  eps=1e-6.

Up to the (negligible, ~3e-6 relative) effect of eps this equals the
numerically stable form
    bce = softplus(x) - t*x  = ln(1+e^x) - t*x
so per element the kernel computes Exp then Ln(1+e) on the scalar engine
(one activation-table load: both live in natural_log_exp_and_others) and
a fused multiply+reduce of t*x on the vector engine.

Inputs are streamed as bf16 (loss-mean error ~1e-7 relative on top of
the 3.5e-6 softplus-identity error; measured total ~3.6e-6) which halves
HBM traffic - this is a memory-bound kernel.

Sharding: pure data parallel - batch 32 split as 4 images per core over
8 NeuronCores.  Each core reduces its shard to a single scalar on-device
(per-partition accumulators -> PE dot with a ones vector -> one 4-byte
output DMA); the host adds the 8 scalars and divides by the element
count.
"""

import contextlib

import numpy as np

B, C, H, W = 32, 1, 320, 320
N_CORES = 8
PER_CORE_ELEMS = (B // N_CORES) * C * H * W  # 409600
P = 128
FREE = PER_CORE_ELEMS // P  # 3200
CHUNKS = (800, 1184, 1216)  # uneven: small first chunk starts ACT earlier

_CACHE = {}


def _single_table_patch():
    """Make exp/ln resolvable only via natural_log_exp_and_others so
    Bacc's insert_act_table_loads emits a single ACT_TABLE_LOAD (set
    indices are preserved; only the function->set mapping is narrowed)."""
    import concourse.bacc as bacc_mod
    import concourse.mybir as mybir

    real = bacc_mod.get_activation_tables

    def patched(arch):
        strip = {mybir.ActivationFunctionType.Exp, mybir.ActivationFunctionType.Ln}
        return {
            name: (fns if name == "natural_log_exp_and_others" else fns - strip)
            for name, fns in real(arch).items()
        }

    @contextlib.contextmanager
    def ctx():
        bacc_mod.get_activation_tables = patched
        try:
            yield
        finally:
            bacc_mod.get_activation_tables = real

    return ctx()


def _fuse_all_blocks(nc):
    """Merge all basic blocks, dropping inter-block branches (no sem
    effects; per-engine order preserved).  Avoids sequencer IRAM refetch
    at block boundaries."""
    import concourse.mybir as mybir

    fn = nc.m.functions[0]
    merged = [
        inst
        for b in fn.blocks
        for inst in b.instructions
        if not isinstance(inst, mybir.InstUnconditionalBranch)
    ]
    fn.blocks[0].instructions[:] = merged
    del fn.blocks[1:]


def _trim_epilogue(nc):
    """Drop the final [reset-drain + sem-range-clear + second all-engine
    barrier].  NEFF completion is gated by each engine reaching the end of
    its stream; the out-DMA completion wait on SP is retained.  Repeat
    executions of the loaded NEFF stay correct (validated on HW)."""
    import concourse.mybir as mybir

    insts = nc.m.functions[0].blocks[0].instructions
    for i, inst in enumerate(insts):
        if isinstance(inst, mybir.InstDrain) and getattr(inst, "is_reset_sema", False):
            del insts[i:]
            break


def _drop_extra_table_loads(nc):
    """Bacc emits a useless set-0 LoadActFuncSet before the set-6 load the
    Exp/Ln chain actually needs; dropping it frees ~1.3us of ACT-sequencer
    time in the critical prefix (validated numerically on HW)."""
    import concourse.mybir as mybir

    insts = nc.m.functions[0].blocks[0].instructions
    for i, inst in reversed(list(enumerate(insts))):
        if (
            isinstance(inst, mybir.InstLoadActFuncSet)
            and inst.act_func_set_id != 6
            and not (inst.sync_info and (inst.sync_info.on_wait or inst.sync_info.on_update))
        ):
            del insts[i]


def _build_nc():
    import concourse.bacc as bacc
    import concourse.mybir as mybir
    import concourse.tile as tile

    f32 = mybir.dt.float32
    bf16 = mybir.dt.bfloat16
    AF = mybir.ActivationFunctionType
    ALU = mybir.AluOpType
    AX = mybir.AxisListType

    nc = bacc.Bacc("TRN2", target_bir_lowering=False)
    x = nc.dram_tensor("x", [P, FREE], bf16, kind="ExternalInput").ap()
    t = nc.dram_tensor("t", [P, FREE], bf16, kind="ExternalInput").ap()
    out = nc.dram_tensor("partial", [1, 1], f32, kind="ExternalOutput").ap()
    x_queues = [nc.sync, nc.scalar, nc.sync]
    t_queues = [nc.gpsimd, nc.gpsimd, nc.gpsimd]

    with tile.TileContext(nc) as tc:
        with (
            tc.tile_pool(name="xin", bufs=1) as xin,
            tc.tile_pool(name="tin", bufs=1) as tin,
            tc.tile_pool(name="work", bufs=2) as work,
            tc.tile_pool(name="acc", bufs=1) as accp,
            tc.tile_pool(name="ps", bufs=1, space="PSUM") as psp,
        ):
            n = len(CHUNKS)
            acc_sp = accp.tile([P, n], f32, tag="accsp")
            acc_tx = accp.tile([P, n], f32, tag="acctx")
            ones = accp.tile([P, 1], f32, tag="ones")
            nc.vector.memset(ones[:], 1.0)
            xts, tts = [], []
            off = 0
            for ci, chw in enumerate(CHUNKS):
                xt = xin.tile([P, chw], bf16, tag=f"x{ci}")
                x_queues[ci % len(x_queues)].dma_start(xt[:], x[:, off : off + chw])
                tt = tin.tile([P, chw], bf16, tag=f"t{ci}")
                t_queues[ci % len(t_queues)].dma_start(tt[:], t[:, off : off + chw])
                xts.append(xt)
                tts.append(tt)
                off += chw
            for ci, chw in enumerate(CHUNKS):
                xt, tt = xts[ci], tts[ci]
                # softplus(x) = Ln(1 + Exp(x)); accum_out gives the
                # per-partition chunk sum within the same instruction.
                et = work.tile([P, chw], f32, tag="exp")
                nc.scalar.activation(et[:], xt[:], AF.Exp)
                spt = work.tile([P, chw], f32, tag="sp")
                nc.scalar.activation(
                    spt[:], et[:], AF.Ln, bias=1.0,
                    accum_out=acc_sp[:, ci : ci + 1],
                )
                # acc_tx[:, ci] = per-partition sum of -(t*x); negated here
                # so the final combine is a pure PSUM accumulation.
                txt = work.tile([P, chw], f32, tag="tx")
                nc.vector.scalar_tensor_tensor(
                    out=txt[:], in0=tt[:], scalar=-1.0, in1=xt[:],
                    op0=ALU.mult, op1=ALU.mult,
                    accum_out=acc_tx[:, ci : ci + 1],
                )
            # Cross-partition reduce on the PE: two accumulating matmuls sum
            # both accumulator matrices into one PSUM row (the tx matmul can
            # issue as soon as the last STT lands, ahead of the last Ln);
            # one DVE reduce yields the scalar, and the output DMA is a
            # single contiguous 4-byte descriptor instead of 128 tiny ones.
            pt = psp.tile([1, n], f32, tag="pt")
            nc.tensor.matmul(pt[:], ones[:], acc_tx[:], start=True, stop=False)
            nc.tensor.matmul(pt[:], ones[:], acc_sp[:], start=False, stop=True)
            sc = accp.tile([1, 1], f32, tag="scout")
            nc.vector.reduce_sum(sc[:], pt[:], axis=AX.X)
            nc.sync.dma_start(out, sc[:])
    with _single_table_patch():
        nc.compile()
    _fuse_all_blocks(nc)
    _trim_epilogue(nc)
    _drop_extra_table_loads(nc)
    return nc


def _get_nc():
    if "nc" not in _CACHE:
        _CACHE["nc"] = _build_nc()
    return _CACHE["nc"]


def _make_in_maps(inputs, targets):
    import ml_dtypes

    bf16 = ml_dtypes.bfloat16
    x = np.ascontiguousarray(inputs, dtype=np.float32).reshape(
        N_CORES, P, FREE
    ).astype(bf16)
    t = np.ascontiguousarray(targets, dtype=np.float32).reshape(
        N_CORES, P, FREE
    ).astype(bf16)
    return [{"x": x[i], "t": t[i]} for i in range(N_CORES)]


def run(inputs, targets, **spmd_kwargs):
    """Run on the 8 NeuronCores; returns (loss, BassKernelResults)."""
    from concourse.bass_utils import run_bass_kernel_spmd

    nc = _get_nc()
    in_maps = _make_in_maps(inputs, targets)
    res = run_bass_kernel_spmd(nc, in_maps, list(range(N_CORES)), **spmd_kwargs)
    total = 0.0
    for r in res.results:
        total += r["partial"].astype(np.float64).sum()
    loss = np.float32(total / (B * C * H * W))
    return loss, res


def kernel(inputs, targets):
    loss, _ = run(inputs, targets)
    return loss
